# revision 12
# baseline (speedup 1.0000x reference)
"""Cross-attention Trainium2 Bass kernel.

Data-parallel over batch: 8 batch elements -> 8 NeuronCores. Each core runs
the full cross-attention for one batch element:
    Q = x @ Wq + bq            [4096, 1024]
    K = y @ Wk + bk, V = y @ Wv + bv     [77, 1024]
    attn = softmax(Q K^T / 8) per head (16 heads, d=64)
    out = (attn @ V) @ Wo + bo  [4096, 1024]

Layout strategy: everything flows "transposed" (feature on partitions,
tokens on the free dim) so no operand of any matmul ever needs an extra
transpose beyond the initial PE-transpose of x tiles:
    xT [E, tok] -> QT[f, tok] -> scoresT[kv, tok] -> expT -> ctxT[E, tok]
    -> out[tok, f] (the final matmul flips back, bias via rank-1 matmul).
Softmax runs with kv on partitions: denominators via accumulating
"sum-select" matmuls into one [16, tok] PSUM tile, one reciprocal, then a
sel-matmul broadcasts recips across partitions for the ctxT normalize.
"""

import sys

import numpy as np

try:
    import concourse.bass as bass  # noqa: F401
except ImportError:  # pragma: no cover
    sys.path.insert(0, "/opt/trn_rl_repo")

import ml_dtypes

import concourse.tile as tile
from concourse import bacc, bass_utils, mybir

F32 = mybir.dt.float32
F32R = mybir.dt.float32r
BF16 = mybir.dt.bfloat16
FP16 = mybir.dt.float16

B, SX, SY = 8, 4096, 77
E, EY, H, D = 1024, 768, 16, 64
NCORES = 8

# ---- tunables -------------------------------------------------------------
CHUNK = 512          # tokens per pipeline chunk (free dim of the T layout)
PROJ_F32R = True     # use float32r (full-rate fp32) for the big matmuls
EXP_DT = "fp16"      # attention weights (exp scores) and V: bf16|fp16|f32
QT_BUFS = 1
XC_BUFS = 3          # shared xT/ctxT pool
EXP_BUFS = 1
# ---------------------------------------------------------------------------

EO = E // 128        # 8  E-chunks / f-tiles
EYO = EY // 128      # 6  Ey-chunks
HPAIRS = H // 2      # 8


# dtype for tensors that feed the big matmuls: float32r operands must be
# *produced* as float32r (the BIR verifier enforces rounded producers), so
# the SBUF tiles and the DRAM declarations carry the dtype end to end.
DT32 = F32R if PROJ_F32R else F32


def build(sx=SX, chunk=CHUNK):
    """Build and compile the Bass module for one core."""
    nchunks = sx // chunk
    tsub = chunk // 128  # token subtiles per chunk

    nc = bacc.Bacc(
        "TRN2",
        target_bir_lowering=False,
        debug=False,
        enable_asserts=False,
        num_devices=NCORES,
    )

    x_d = nc.dram_tensor("x", [sx, E], F32, kind="ExternalInput")
    y_d = nc.dram_tensor("y", [SY, EY], F32, kind="ExternalInput")
    wq_d = nc.dram_tensor("Wq", [E, E], DT32, kind="ExternalInput")
    wk_d = nc.dram_tensor("Wk", [EY, E], F32, kind="ExternalInput")
    wv_d = nc.dram_tensor("Wv", [EY, E], F32, kind="ExternalInput")
    wo_d = nc.dram_tensor("Wo", [E, E], DT32, kind="ExternalInput")
    bq_d = nc.dram_tensor("bq", [E], F32, kind="ExternalInput")
    bk_d = nc.dram_tensor("bk", [E], F32, kind="ExternalInput")
    # bv is folded into bo on the host: softmax rows sum to 1, so
    # attn @ (V + bv) = attn @ V + bv, and (ctx + bv) @ Wo + bo uses
    # bo_eff = bv @ Wo + bo.
    bo_d = nc.dram_tensor("bo", [1, E], DT32, kind="ExternalInput")
    id_d = nc.dram_tensor("ident", [128, 128], F32, kind="ExternalInput")
    ones_d = nc.dram_tensor("onesrow", [1, 128], DT32, kind="ExternalInput")
    sel_d = nc.dram_tensor("sel", [16, E], DT32, kind="ExternalInput")
    edt = {"bf16": BF16, "fp16": FP16, "f32": F32}[EXP_DT]
    ssel_d = nc.dram_tensor("sumsel", [SY, 16 * 16], edt, kind="ExternalInput")
    out_d = nc.dram_tensor("out", [sx, E], F32, kind="ExternalOutput")

    xr = x_d.ap().rearrange("(t p) e -> t p e", p=128)       # [sx/128, 128, E]
    outr = out_d.ap().rearrange("(t p) e -> t p e", p=128)
    wqr = wq_d.ap().rearrange("(o p) f -> p o f", p=128)     # [128, 8, E]
    wor = wo_d.ap().rearrange("(o p) f -> p o f", p=128)
    wkr = wk_d.ap().rearrange("(o p) f -> p o f", p=128)     # [128, 6, E]
    wvr = wv_d.ap().rearrange("(o p) f -> p o f", p=128)

    with tile.TileContext(nc) as tc:
        from contextlib import ExitStack

        with ExitStack() as ctx:
            const = ctx.enter_context(tc.tile_pool(name="const", bufs=1))
            stream = ctx.enter_context(tc.tile_pool(name="stream", bufs=2))
            qtp = ctx.enter_context(tc.tile_pool(name="qtp", bufs=QT_BUFS))
            xcp = ctx.enter_context(tc.tile_pool(name="xcp", bufs=XC_BUFS))
            expp = ctx.enter_context(tc.tile_pool(name="expp", bufs=EXP_BUFS))
            smalls = ctx.enter_context(tc.tile_pool(name="smalls", bufs=2))
            outp = ctx.enter_context(tc.tile_pool(name="outp", bufs=2))
            ps_tr = ctx.enter_context(tc.tile_pool(name="ps_tr", bufs=1, space="PSUM"))
            ps_qt = ctx.enter_context(tc.tile_pool(name="ps_qt", bufs=1, space="PSUM"))
            ps_sc = ctx.enter_context(tc.tile_pool(name="ps_sc", bufs=1, space="PSUM"))
            ps_sm = ctx.enter_context(tc.tile_pool(name="ps_sm", bufs=1, space="PSUM"))
            ps_rb = ctx.enter_context(tc.tile_pool(name="ps_rb", bufs=1, space="PSUM"))
            ps_cx = ctx.enter_context(tc.tile_pool(name="ps_cx", bufs=1, space="PSUM"))
            ps_ou = ctx.enter_context(tc.tile_pool(name="ps_ou", bufs=1, space="PSUM"))

            # ---- constants / weights resident in SBUF ----
            ident = const.tile([128, 128], F32)
            nc.sync.dma_start(ident[:], id_d.ap())
            onesrow = const.tile([1, 128], DT32)
            nc.sync.dma_start(onesrow[:], ones_d.ap())
            sel = const.tile([16, E], DT32)
            nc.sync.dma_start(sel[:], sel_d.ap())
            ssel = const.tile([SY, 16 * 16], edt)
            nc.sync.dma_start(ssel[:], ssel_d.ap())
            bqs = const.tile([128, EO], F32)
            nc.sync.dma_start(bqs[:], bq_d.ap().rearrange("(o p) -> p o", p=128))
            bks = const.tile([128, EO], F32)
            nc.sync.dma_start(bks[:], bk_d.ap().rearrange("(o p) -> p o", p=128))
            bos = const.tile([1, E], DT32)
            nc.sync.dma_start(bos[:], bo_d.ap())
            wq = const.tile([128, EO, E], DT32)
            nc.sync.dma_start(wq[:], wqr)
            wo = const.tile([128, EO, E], DT32)
            nc.sync.dma_start(wo[:], wor)

            # ---- y path: yT, KT, V (tiny, once) ----
            y_in = stream.tile([SY, EY], F32, tag="xin")
            nc.sync.dma_start(y_in[:], y_d.ap())
            yt = const.tile([128, EYO, SY], F32)
            for c in range(EYO):
                ptr = ps_tr.tile([128, SY], F32, tag="tr")
                nc.tensor.transpose(ptr[:], y_in[:, c * 128:(c + 1) * 128], ident[:SY, :SY])
                nc.vector.tensor_copy(yt[:, c, :], ptr[:])

            kt = const.tile([128, EO, SY], DT32)
            for f in range(EO):
                wks = stream.tile([128, EYO, 128], F32, tag="wstream")
                nc.sync.dma_start(wks[:], wkr[:, :, f * 128:(f + 1) * 128])
                pkt = ps_tr.tile([128, SY], F32, tag="tr")
                for c in range(EYO):
                    nc.tensor.matmul(
                        pkt[:], wks[:, c, :], yt[:, c, :],
                        start=(c == 0), stop=(c == EYO - 1),
                    )
                # bias bk (per-partition) fused into the PSUM->SBUF copy
                nc.scalar.activation(
                    kt[:, f, :], pkt[:],
                    mybir.ActivationFunctionType.Identity,
                    bias=bks[:, f:f + 1],
                )

            vsb = const.tile([SY, E], edt)
            for q in range(EO):
                wvs = stream.tile([128, EYO, 128], F32, tag="wstream")
                nc.sync.dma_start(wvs[:], wvr[:, :, q * 128:(q + 1) * 128])
                pv = ps_qt.tile([SY, 128], F32, tag="qt")
                for c in range(EYO):
                    nc.tensor.matmul(
                        pv[:], yt[:, c, :], wvs[:, c, :],
                        start=(c == 0), stop=(c == EYO - 1),
                    )
                nc.vector.tensor_copy(vsb[:, q * 128:(q + 1) * 128], pv[:])

            # ---- main chunk loop ----
            for ch in range(nchunks):
                t0 = ch * tsub

                xts = xcp.tile([128, EO, chunk], DT32, tag="xc")
                for t in range(tsub):
                    xin = stream.tile([128, E], F32, tag="xin")
                    nc.sync.dma_start(xin[:], xr[t0 + t])
                    for e in range(EO):
                        ptr = ps_tr.tile([128, 128], F32, tag="tr")
                        nc.tensor.transpose(
                            ptr[:], xin[:, e * 128:(e + 1) * 128], ident[:]
                        )
                        nc.vector.tensor_copy(
                            xts[:, e, t * 128:(t + 1) * 128], ptr[:]
                        )

                qt = qtp.tile([128, EO, chunk], DT32, tag="qt")
                for f in range(EO):
                    pq = ps_qt.tile([128, chunk], F32, tag="qt")
                    for e in range(EO):
                        nc.tensor.matmul(
                            pq[:], wq[:, e, f * 128:(f + 1) * 128],
                            xts[:, e, :],
                            start=(e == 0), stop=(e == EO - 1),
                        )
                    nc.scalar.activation(
                        qt[:, f, :], pq[:],
                        mybir.ActivationFunctionType.Identity,
                        bias=bqs[:, f:f + 1],
                    )

                expt = expp.tile([SY, H, chunk], edt, tag="exp")
                for p in range(HPAIRS):
                    psc = ps_sc.tile([SY, 2, chunk], F32, tag="sc")
                    for hh in range(2):
                        nc.tensor.matmul(
                            psc[:, hh, :],
                            kt[hh * 64:(hh + 1) * 64, p, :],
                            qt[hh * 64:(hh + 1) * 64, p, :],
                            start=True, stop=True,
                        )
                    nc.scalar.activation(
                        expt[:, 2 * p:2 * p + 2, :], psc[:],
                        mybir.ActivationFunctionType.Exp,
                        scale=0.125,
                    )

                psm = ps_sm.tile([16, chunk], F32, tag="sm")
                for h in range(H):
                    nc.tensor.matmul(
                        psm[:], ssel[:, h * 16:(h + 1) * 16], expt[:, h, :],
                        start=(h == 0), stop=(h == H - 1),
                    )
                recip = smalls.tile([16, chunk], DT32, tag="recip")
                with nc.allow_low_precision(reason="recip rounded to f32r for PE broadcast"):
                    nc.vector.reciprocal(recip[:], psm[:])

                cxt = xcp.tile([128, EO, chunk], DT32, tag="xc")
                for e in range(EO):
                    prb = ps_rb.tile([128, chunk], F32, tag="rb")
                    nc.tensor.matmul(
                        prb[:], sel[:, e * 128:(e + 1) * 128], recip[:],
                        start=True, stop=True,
                    )
                    # DVE may read only one PSUM operand -> stage rbc in SBUF
                    # via the (lightly loaded) scalar engine.
                    rbc = smalls.tile([128, chunk], F32, tag="rbc")
                    nc.scalar.copy(rbc[:], prb[:])
                    pcx = ps_cx.tile([128, chunk], F32, tag="cx")
                    for hh in range(2):
                        h = 2 * e + hh
                        nc.tensor.matmul(
                            pcx[hh * 64:(hh + 1) * 64, :],
                            vsb[:, h * 64:(h + 1) * 64], expt[:, h, :],
                            start=True, stop=True,
                            tile_position=(0, hh * 64),
                        )
                    nc.vector.tensor_tensor(
                        cxt[:, e, :], pcx[:], rbc[:], mybir.AluOpType.mult
                    )

                for t in range(tsub):
                    osb = outp.tile([128, E], F32, tag="out")
                    for fh in range(2):
                        po = ps_ou.tile([128, 512], F32, tag="ou")
                        for e in range(EO):
                            nc.tensor.matmul(
                                po[:],
                                cxt[:, e, t * 128:(t + 1) * 128],
                                wo[:, e, fh * 512:(fh + 1) * 512],
                                start=(e == 0), stop=False,
                            )
                        nc.tensor.matmul(
                            po[:], onesrow[:],
                            bos[:, fh * 512:(fh + 1) * 512],
                            start=False, stop=True,
                        )
                        nc.vector.tensor_copy(osb[:, fh * 512:(fh + 1) * 512], po[:])
                    nc.sync.dma_start(outr[t0 + t], osb[:])

    nc.compile()
    return nc


def make_aux():
    ident = np.eye(128, dtype=np.float32)
    onesrow = np.ones((1, 128), np.float32)
    sel = np.zeros((16, E), np.float32)
    for e in range(EO):
        sel[2 * e, e * 128:e * 128 + 64] = 1.0
        sel[2 * e + 1, e * 128 + 64:(e + 1) * 128] = 1.0
    ssel = np.zeros((SY, 16, 16), np.float32)
    for h in range(16):
        ssel[:, h, h] = 1.0
    ssel = ssel.reshape(SY, 256)
    ssel = ssel.astype(
        {"bf16": ml_dtypes.bfloat16, "fp16": np.float16, "f32": np.float32}[EXP_DT]
    )
    return ident, onesrow, sel, ssel


def make_in_maps(x, y, Wq, bq, Wk, bk, Wv, bv, Wo, bo, sx=SX):
    ident, onesrow, sel, ssel = make_aux()
    shared = {
        "Wq": np.ascontiguousarray(Wq, np.float32),
        "Wk": np.ascontiguousarray(Wk, np.float32),
        "Wv": np.ascontiguousarray(Wv, np.float32),
        "Wo": np.ascontiguousarray(Wo, np.float32),
        "bq": np.ascontiguousarray(bq, np.float32),
        "bk": np.ascontiguousarray(bk, np.float32),
        "bo": (np.asarray(bv, np.float64) @ np.asarray(Wo, np.float64)
               + np.asarray(bo, np.float64)).astype(np.float32).reshape(1, E),
        "ident": ident, "onesrow": onesrow, "sel": sel, "sumsel": ssel,
    }
    return [
        {
            "x": np.ascontiguousarray(x[b, :sx], np.float32),
            "y": np.ascontiguousarray(y[b], np.float32),
            **shared,
        }
        for b in range(NCORES)
    ]


_nc_cache = {}


def get_nc(sx=SX, chunk=CHUNK):
    key = (sx, chunk, PROJ_F32R, EXP_DT)
    if key not in _nc_cache:
        _nc_cache[key] = build(sx, chunk)
    return _nc_cache[key]


def kernel(x, y, Wq, bq, Wk, bk, Wv, bv, Wo, bo):
    nc = get_nc()
    in_maps = make_in_maps(x, y, Wq, bq, Wk, bk, Wv, bv, Wo, bo)
    res = bass_utils.run_bass_kernel_spmd(nc, in_maps, core_ids=list(range(NCORES)))
    return np.stack([res.results[b]["out"] for b in range(NCORES)]).astype(np.float32)
